# revision 30
# baseline (speedup 1.0000x reference)
"""Trainium2 Bass kernel for nn_Net_34729105555716.

Model: embedding lookup [30000,100] -> input projection (w_ih) -> 200-step
tanh RNN (hidden 300) -> relu MLP (300->256->3) over batch 4096.

Strategy (data-parallel over batch, 512 rows per core, 8 cores):
  - The fp16 embedding table lives in SBUF in dma_gather's
    tokens_per_rank=128 layout (partition = token%128, rank = token//128,
    rows padded 100->128 elems = 256B).
  - Per RNN step, a transpose-mode SWDGE dma_gather pulls the 512 tokens'
    embedding rows directly into matmul rhs layout [emb_dim(partitions),
    batch(free)], batched 2 steps per gather instruction.
  - PE does 12 fp16 matmuls per step (3 input-projection + 9 recurrent,
    K-tiles 128/128/44, M-tiles 128/128/44, N=512) accumulating in PSUM.
  - ScalarE applies tanh with the per-hidden bias (b_ih+b_hh) fused,
    writing the fp16 hidden state for the next step.
  - MLP head: 6 + 2 matmuls, relu fused on ScalarE, fc2 bias on VectorE.
Host side only marshals inputs (dtype cast, transpose, index layout) and
transposes the [3,512] per-core outputs back to [4096,3].
"""

import sys

if "/opt/trn_rl_repo" not in sys.path:
    sys.path.insert(0, "/opt/trn_rl_repo")

import numpy as np

SEQ = 200
BATCH = 4096
VOCAB = 30000
EMB = 100
HID = 300
FC1 = 256
N_CORES = 8
BPC = BATCH // N_CORES  # batch per core
G = 1  # RNN steps per dma_gather instruction (>512 idxs per gather crashes)
N_RANKS = (VOCAB + 127) // 128  # 235
KT = [(0, 128), (128, 128), (256, 44)]  # hidden-dim tiles

_cached = {}


def _split_multiwait(nc, mybir):
    """walrus in this container rejects >1 embedded sync wait per
    instruction (>2 for EventSemaphore); split extras onto NoOp carriers."""
    n = 0
    for f in nc.m.functions:
        for blk in f.blocks:
            if not any(
                i.sync_info is not None and len(i.sync_info.on_wait) > 1
                for i in blk.instructions
            ):
                continue
            out = []
            for inst in blk.instructions:
                si = inst.sync_info
                cap = 2 if isinstance(inst, mybir.InstEventSemaphore) else 1
                if si is not None and len(si.on_wait) > cap:
                    waits = list(si.on_wait)
                    for w in waits[:-cap]:
                        n += 1
                        carrier = mybir.InstNoOp(
                            name=f"I-waitsplit-{n}", ins=[], outs=[]
                        )
                        carrier.engine = inst.engine
                        carrier.sync_info = mybir.SyncInfo(
                            on_wait=[w], on_update=[]
                        )
                        out.append(carrier)
                    si.on_wait = waits[-cap:]
                out.append(inst)
            blk.instructions = out
    return n


def _build(seq=SEQ, split_multiwait=True, no_gather=False, mouter=False):
    import concourse.bass as bass
    import concourse.mybir as mybir
    import concourse.tile as tile
    from concourse import library_config
    from concourse.tile import add_dep_helper

    dt = mybir.dt
    f16, f32, i16 = dt.float16, dt.float32, dt.int16
    Tanh = mybir.ActivationFunctionType.Tanh
    Relu = mybir.ActivationFunctionType.Relu

    nc = bass.Bass(
        "TRN2", target_bir_lowering=False, debug=False, num_devices=N_CORES
    )
    x_idx = nc.dram_tensor(
        "x_idx", [128, seq * BPC // 16], i16, kind="ExternalInput"
    )
    emb_sb = nc.dram_tensor(
        "emb_sb", [128, N_RANKS * 128], f16, kind="ExternalInput"
    )
    whh_t = nc.dram_tensor("whh_t", [HID, HID], f16, kind="ExternalInput")
    wih_t = nc.dram_tensor("wih_t", [EMB, HID], f16, kind="ExternalInput")
    fc1_t = nc.dram_tensor("fc1_t", [HID, FC1], f16, kind="ExternalInput")
    fc2_t = nc.dram_tensor("fc2_t", [FC1, 3], f16, kind="ExternalInput")
    bias_sb = nc.dram_tensor("bias_sb", [128, 3], f32, kind="ExternalInput")
    fc1b_sb = nc.dram_tensor("fc1b_sb", [128, 2], f32, kind="ExternalInput")
    fc2b_sb = nc.dram_tensor("fc2b_sb", [3, 1], f32, kind="ExternalInput")
    out = nc.dram_tensor("out", [3, BPC], f32, kind="ExternalOutput")

    with tile.TileContext(nc) as tc:
        with (
            tc.tile_pool(name="const", bufs=1) as cpool,
            tc.tile_pool(name="gather", bufs=4) as gpool,
            tc.tile_pool(name="h", bufs=2) as hpool,
            tc.tile_pool(name="psum", bufs=2, space="PSUM") as ppool,
        ):
            lib_inst = nc.gpsimd.load_library(library_config.mlp)

            tbl = cpool.tile([128, N_RANKS * 128], f16, tag="tbl")
            nc.sync.dma_start(tbl[:], emb_sb.ap())
            idx = cpool.tile([128, seq * BPC // 16], i16, tag="idx")
            nc.sync.dma_start(idx[:], x_idx.ap())

            whh = []
            for o, sz in KT:
                w = cpool.tile([sz, HID], f16, tag=f"whh{o}")
                nc.sync.dma_start(w[:], whh_t.ap()[o : o + sz, :])
                whh.append(w)
            wih = cpool.tile([EMB, HID], f16, tag="wih")
            nc.sync.dma_start(wih[:], wih_t.ap())
            fc1 = []
            for o, sz in KT:
                w = cpool.tile([sz, FC1], f16, tag=f"fc1{o}")
                nc.sync.dma_start(w[:], fc1_t.ap()[o : o + sz, :])
                fc1.append(w)
            fc2 = []
            for o in (0, 128):
                w = cpool.tile([128, 3], f16, tag=f"fc2{o}")
                nc.sync.dma_start(w[:], fc2_t.ap()[o : o + 128, :])
                fc2.append(w)
            bias_t = cpool.tile([128, 3], f32, tag="bias")
            nc.sync.dma_start(bias_t[:], bias_sb.ap())
            fc1b_t = cpool.tile([128, 2], f32, tag="fc1b")
            nc.sync.dma_start(fc1b_t[:], fc1b_sb.ap())
            fc2b_t = cpool.tile([3, 1], f32, tag="fc2b")
            nc.sync.dma_start(fc2b_t[:], fc2b_sb.ap())

            reg_n = nc.gpsimd.to_reg(G * BPC)

            h = []
            for mi, (o, sz) in enumerate(KT):
                ht = hpool.tile([sz, BPC], f16, tag=f"h{mi}")
                nc.vector.memset(ht[:], 0)
                h.append(ht)

            xg = None
            if no_gather:
                xg_fixed = cpool.tile([128, 1, G * BPC], f16, tag="xe_fixed")
                nc.vector.memset(xg_fixed[:], 0)
            for t in range(seq):
                g, sl = divmod(t, G)
                if no_gather:
                    xg = xg_fixed
                elif sl == 0:
                    n_idx = min(G, seq - t) * BPC
                    xg = gpool.tile([128, 1, G * BPC], f16, tag="xe")
                    gi = nc.gpsimd.dma_gather(
                        xg[:, :, :n_idx],
                        tbl[:],
                        idx[:, g * (G * BPC // 16) : g * (G * BPC // 16) + n_idx // 16],
                        n_idx,
                        reg_n,
                        128,
                        transpose=True,
                        sbuf_tokens_per_rank=128,
                        sbuf_free_dim_per_rank=256,
                    )
                    add_dep_helper(
                        gi.ins, lib_inst.ins, sync=False, reason="lib first"
                    )
                xe = xg[0:EMB, 0, sl * BPC : (sl + 1) * BPC]
                if mouter:
                    # M-outer: each psum finishes early (slots 4/8/12), so
                    # each tanh gets pipeline slack before the next step
                    # needs its h tile
                    hn = []
                    for mi, (o, sz) in enumerate(KT):
                        p = ppool.tile([sz, BPC], f32, tag=f"p{mi}")
                        nc.tensor.matmul(
                            p[:], wih[:, o : o + sz], xe, start=True, stop=False
                        )
                        for ki, (ko, ks) in enumerate(KT):
                            nc.tensor.matmul(
                                p[:],
                                whh[ki][:, o : o + sz],
                                h[ki][:],
                                start=False,
                                stop=(ki == len(KT) - 1),
                            )
                        ht = hpool.tile([sz, BPC], f16, tag=f"h{mi}")
                        nc.scalar.activation(
                            ht[:], p[:], Tanh, bias=bias_t[0:sz, mi : mi + 1]
                        )
                        hn.append(ht)
                    h = hn
                    continue
                ps = []
                for mi, (o, sz) in enumerate(KT):
                    p = ppool.tile([sz, BPC], f32, tag=f"p{mi}")
                    nc.tensor.matmul(
                        p[:], wih[:, o : o + sz], xe, start=True, stop=False
                    )
                    ps.append(p)
                for ki, (ko, ks) in enumerate(KT):
                    last = ki == len(KT) - 1
                    for mi, (mo, ms) in enumerate(KT):
                        nc.tensor.matmul(
                            ps[mi][:],
                            whh[ki][:, mo : mo + ms],
                            h[ki][:],
                            start=False,
                            stop=last,
                        )
                hn = []
                for mi, (o, sz) in enumerate(KT):
                    ht = hpool.tile([sz, BPC], f16, tag=f"h{mi}")
                    nc.scalar.activation(
                        ht[:], ps[mi][:], Tanh, bias=bias_t[0:sz, mi : mi + 1]
                    )
                    hn.append(ht)
                h = hn

            # MLP head
            h1 = []
            for mi in range(2):
                p = ppool.tile([128, BPC], f32, tag=f"p{mi}")
                for ki, (ko, ks) in enumerate(KT):
                    nc.tensor.matmul(
                        p[:],
                        fc1[ki][:, mi * 128 : (mi + 1) * 128],
                        h[ki][:],
                        start=(ki == 0),
                        stop=(ki == len(KT) - 1),
                    )
                ht = hpool.tile([128, BPC], f16, tag=f"h1_{mi}")
                nc.scalar.activation(
                    ht[:], p[:], Relu, bias=fc1b_t[:, mi : mi + 1]
                )
                h1.append(ht)
            p2 = ppool.tile([3, BPC], f32, tag="p2")
            nc.tensor.matmul(p2[:], fc2[0][:, :], h1[0][:], start=True, stop=False)
            nc.tensor.matmul(p2[:], fc2[1][:, :], h1[1][:], start=False, stop=True)
            osb = cpool.tile([3, BPC], f32, tag="osb")
            nc.vector.tensor_scalar_add(osb[:], p2[:], fc2b_t[:, 0:1])
            nc.sync.dma_start(out.ap(), osb[:])

    # Populate .instr bytes for InstISA subclasses (library reload etc.) —
    # Bacc.compile does this; raw Bass+Tile must do it explicitly or walrus
    # fails with "ISA wrong length".
    mybir.codegen_inst_isa_subclasses(nc)
    if split_multiwait:
        _split_multiwait(nc, mybir)
    return nc


def _build_v2(seq=SEQ, split_multiwait=True, no_gather=False, pack_k2=False,
              gq=1, probe=None, fold=False):
    """v2: pre-project the embedding table on device (proj = emb @ w_ih.T,
    [VOCAB, 300] fp16 in SWDGE gather layout), then each RNN step gathers the
    projected rows directly — the 3 input-projection matmuls per step are
    gone (12 -> 9 MMs). M-outer accumulation order gives each tanh ~3 MM
    slots of pipeline slack. Gathered projection is added into PSUM by DVE.
    """
    import concourse.bass as bass
    import concourse.mybir as mybir
    import concourse.tile as tile
    from concourse import library_config
    from concourse.tile import add_dep_helper

    dt = mybir.dt
    f16, f32, i16 = dt.float16, dt.float32, dt.int16
    Tanh = mybir.ActivationFunctionType.Tanh
    Relu = mybir.ActivationFunctionType.Relu
    Copy = mybir.ActivationFunctionType.Copy
    STRIPE = 384  # fp16 elems per proj rank stripe (300 used + 84 pad), 768B
    IC = 16  # idx chunk: steps per streamed idx tile

    nc = bass.Bass(
        "TRN2", target_bir_lowering=False, debug=False, num_devices=N_CORES,
        num_swdge_queues=gq,
    )
    x_idx = nc.dram_tensor(
        "x_idx", [128, seq * BPC // 16], i16, kind="ExternalInput"
    )
    embT = nc.dram_tensor("embT", [EMB, VOCAB], f16, kind="ExternalInput")
    whh_t = nc.dram_tensor("whh_t", [HID, HID], f16, kind="ExternalInput")
    wih_t = nc.dram_tensor("wih_t", [EMB, HID], f16, kind="ExternalInput")
    fc1_t = nc.dram_tensor("fc1_t", [HID, FC1], f16, kind="ExternalInput")
    fc2_t = nc.dram_tensor("fc2_t", [FC1, 3], f16, kind="ExternalInput")
    bias_sb = nc.dram_tensor("bias_sb", [128, 3], f32, kind="ExternalInput")
    fc1b_sb = nc.dram_tensor("fc1b_sb", [128, 2], f32, kind="ExternalInput")
    fc2b_sb = nc.dram_tensor("fc2b_sb", [3, 1], f32, kind="ExternalInput")
    ident = None
    if fold:
        ident = nc.dram_tensor("ident", [128, 128], f16, kind="ExternalInput")
    out = nc.dram_tensor("out", [3, BPC], f32, kind="ExternalOutput")

    n_chunks = (VOCAB + 127) // 128  # 235, last chunk 48 tokens

    with tile.TileContext(nc) as tc:
        with (
            tc.tile_pool(name="const", bufs=1) as cpool,
            tc.tile_pool(name="estream", bufs=2) as epool,
            tc.tile_pool(name="idxs", bufs=2) as ipool,
            tc.tile_pool(name="gather", bufs=2) as gpool,
            tc.tile_pool(name="h", bufs=2) as hpool,
            tc.tile_pool(name="psum", bufs=2, space="PSUM") as ppool,
            tc.tile_pool(name="pprep", bufs=2, space="PSUM") as prep_pool,
        ):
            lib_inst = nc.gpsimd.load_library(library_config.mlp)

            whh = []
            for o, sz in KT:
                w = cpool.tile([sz, HID], f16, tag=f"whh{o}")
                nc.sync.dma_start(w[:], whh_t.ap()[o : o + sz, :])
                whh.append(w)
            wih = cpool.tile([EMB, HID], f16, tag="wih")
            nc.sync.dma_start(wih[:], wih_t.ap())
            fc1 = []
            for o, sz in KT:
                w = cpool.tile([sz, FC1], f16, tag=f"fc1{o}")
                nc.sync.dma_start(w[:], fc1_t.ap()[o : o + sz, :])
                fc1.append(w)
            fc2 = []
            for o in (0, 128):
                w = cpool.tile([128, 3], f16, tag=f"fc2{o}")
                nc.sync.dma_start(w[:], fc2_t.ap()[o : o + 128, :])
                fc2.append(w)
            bias_t = cpool.tile([128, 3], f32, tag="bias")
            nc.sync.dma_start(bias_t[:], bias_sb.ap())
            fc1b_t = cpool.tile([128, 2], f32, tag="fc1b")
            nc.sync.dma_start(fc1b_t[:], fc1b_sb.ap())
            fc2b_t = cpool.tile([3, 1], f32, tag="fc2b")
            nc.sync.dma_start(fc2b_t[:], fc2b_sb.ap())
            if fold:
                ident_t = cpool.tile([128, 128], f16, tag="ident")
                nc.sync.dma_start(ident_t[:], ident.ap())

            # ---- phase A: project the embedding table ----
            proj = cpool.tile([128, n_chunks, STRIPE], f16, tag="proj")
            # zero the stripe pads (gather copies whole 768B rows) and the
            # tail of the last chunk (tokens 29952..30080 don't all exist)
            nc.vector.memset(proj[:, :, HID:STRIPE], 0)
            nc.vector.memset(proj[32:64, n_chunks - 1, 0:HID], 0)
            nc.vector.memset(proj[64:128, n_chunks - 1, 0:HID], 0)
            CH = 4  # table chunks per DMA load
            for c0 in range(0, n_chunks, CH):
                nch = min(CH, n_chunks - c0)
                ncol = min(nch * 128, VOCAB - c0 * 128)
                ech = epool.tile([EMB, CH * 128], f16, tag="ech")
                nc.sync.dma_start(
                    ech[:, :ncol], embT.ap()[:, c0 * 128 : c0 * 128 + ncol]
                )
                for c in range(c0, c0 + nch):
                    ntok = min(128, VOCAB - c * 128)
                    off = (c - c0) * 128
                    pp = prep_pool.tile([128, HID], f32, tag="pp")
                    nc.tensor.matmul(
                        pp[:ntok, :],
                        ech[:, off : off + ntok],
                        wih[:],
                        start=True,
                        stop=True,
                    )
                    dst = proj[:ntok, c, 0:HID]
                    if c % 2 == 0:
                        nc.vector.tensor_copy(dst, pp[:ntok, :])
                    else:
                        nc.scalar.activation(dst, pp[:ntok, :], Copy)

            reg_n = nc.gpsimd.to_reg(BPC)

            if pack_k2:
                # duplicate of whh k2-rows at partitions 64..107 so the
                # m1/k2 matmul can run in the 64-127 row strips of the PE
                # array concurrently with m0/k2 in rows 0-63
                whh2b_t = cpool.tile([128, HID], f16, tag="whh2b")
                nc.sync.dma_start(
                    whh2b_t[64 : 64 + 44, :], whh_t.ap()[256:300, :]
                )

            h = []
            for mi, (o, sz) in enumerate(KT):
                ht = hpool.tile([sz, BPC], f16, tag=f"h{mi}")
                nc.vector.memset(ht[:], 0)
                h.append(ht)
            h2b = None
            if pack_k2:
                h2b_t = hpool.tile([128, BPC], f16, tag="h2b")
                nc.vector.memset(h2b_t[64 : 64 + 44, :], 0)
                h2b = h2b_t

            if no_gather:
                xg_fixed = cpool.tile([128, 3, BPC], f16, tag="xg_fixed")
                nc.vector.memset(xg_fixed[:], 0)

            # ---- phase B: recurrent steps ----
            idx_t = None
            for t in range(seq):
                ci, sl = divmod(t, IC)
                if sl == 0:
                    nst = min(IC, seq - ci * IC)
                    idx_t = ipool.tile([128, IC * BPC // 16], i16, tag="idx")
                    nc.sync.dma_start(
                        idx_t[:, : nst * BPC // 16],
                        x_idx.ap()[
                            :, ci * IC * BPC // 16 : (ci * IC + nst) * BPC // 16
                        ],
                    )
                if no_gather:
                    xg = xg_fixed
                else:
                    xg = gpool.tile([128, 3, BPC], f16, tag="xg")
                    gi = nc.gpsimd.dma_gather(
                        xg[:],
                        proj[:],
                        idx_t[:, sl * (BPC // 16) : (sl + 1) * (BPC // 16)],
                        BPC,
                        reg_n,
                        STRIPE,
                        transpose=True,
                        sbuf_tokens_per_rank=128,
                        sbuf_free_dim_per_rank=2 * STRIPE,
                        queue_num=t % gq,
                    )
                    add_dep_helper(
                        gi.ins, lib_inst.ins, sync=False, reason="lib first"
                    )
                hn = []
                if pack_k2:
                    # 8 PE slots: m0(k0,k1,k2) with m1k2 packed beside m0k2
                    # (rows 64-127), then m1(k0,k1), then m2(k0,k1,k2)
                    ps = []
                    for mi, (o, sz) in enumerate(KT):
                        p = ppool.tile([sz, BPC], f32, tag=f"p{mi}")
                        ps.append(p)
                    nc.tensor.matmul(
                        ps[0][:], whh[0][:, 0:128], h[0][:],
                        start=True, stop=False,
                    )
                    nc.tensor.matmul(
                        ps[0][:], whh[1][:, 0:128], h[1][:],
                        start=False, stop=False,
                    )
                    nc.tensor.matmul(
                        ps[0][:], whh[2][:, 0:128], h[2][:],
                        start=False, stop=True,
                    )
                    nc.tensor.matmul(
                        ps[1][:], whh2b_t[64 : 64 + 44, 128:256],
                        h2b[64 : 64 + 44, :], start=True, stop=False,
                    )
                    nc.tensor.matmul(
                        ps[1][:], whh[0][:, 128:256], h[0][:],
                        start=False, stop=False,
                    )
                    nc.tensor.matmul(
                        ps[1][:], whh[1][:, 128:256], h[1][:],
                        start=False, stop=True,
                    )
                    for ki in range(3):
                        nc.tensor.matmul(
                            ps[2][:], whh[ki][:, 256:300], h[ki][:],
                            start=(ki == 0), stop=(ki == 2),
                        )
                    for mi, (o, sz) in enumerate(KT):
                        nc.vector.tensor_add(
                            ps[mi][:], ps[mi][:], xg[0:sz, mi, :]
                        )
                        ht = hpool.tile([sz, BPC], f16, tag=f"h{mi}")
                        nc.scalar.activation(
                            ht[:], ps[mi][:], Tanh,
                            bias=bias_t[0:sz, mi : mi + 1],
                        )
                        hn.append(ht)
                    h2b = hpool.tile([128, BPC], f16, tag="h2b")
                    nc.sync.dma_start(h2b[64 : 64 + 44, :], hn[2][:])
                elif probe == "mm":
                    # PE stream only: no adds, no tanh, h stays static
                    for mi, (o, sz) in enumerate(KT):
                        p = ppool.tile([sz, BPC], f32, tag=f"p{mi}")
                        for ki, (ko, ks) in enumerate(KT):
                            nc.tensor.matmul(
                                p[:],
                                whh[ki][:, o : o + sz],
                                h[ki][:],
                                start=(ki == 0),
                                stop=(ki == len(KT) - 1),
                            )
                    hn = h
                elif probe == "mmact":
                    # PE + ScalarE in parallel, but tanh output unused
                    for mi, (o, sz) in enumerate(KT):
                        p = ppool.tile([sz, BPC], f32, tag=f"p{mi}")
                        for ki, (ko, ks) in enumerate(KT):
                            nc.tensor.matmul(
                                p[:],
                                whh[ki][:, o : o + sz],
                                h[ki][:],
                                start=(ki == 0),
                                stop=(ki == len(KT) - 1),
                            )
                        ht = hpool.tile([sz, BPC], f16, tag=f"dump{mi}")
                        nc.scalar.activation(
                            ht[:], p[:], Tanh, bias=bias_t[0:sz, mi : mi + 1]
                        )
                    hn = h
                elif fold:
                    # inject xp into psum via identity matmul (slot 1 of each
                    # M-block, no h dependency) -> tanh reads psum directly;
                    # 2-hop loop chain, DVE stays out of it
                    for mi, (o, sz) in enumerate(KT):
                        p = ppool.tile([sz, BPC], f32, tag=f"p{mi}")
                        nc.tensor.matmul(
                            p[:],
                            ident_t[0:sz, 0:sz],
                            xg[0:sz, mi, :],
                            start=True,
                            stop=False,
                        )
                        for ki, (ko, ks) in enumerate(KT):
                            nc.tensor.matmul(
                                p[:],
                                whh[ki][:, o : o + sz],
                                h[ki][:],
                                start=False,
                                stop=(ki == len(KT) - 1),
                            )
                        ht = hpool.tile([sz, BPC], f16, tag=f"h{mi}")
                        nc.scalar.activation(
                            ht[:], p[:], Tanh, bias=bias_t[0:sz, mi : mi + 1]
                        )
                        hn.append(ht)
                else:
                    for mi, (o, sz) in enumerate(KT):
                        p = ppool.tile([sz, BPC], f32, tag=f"p{mi}")
                        for ki, (ko, ks) in enumerate(KT):
                            nc.tensor.matmul(
                                p[:],
                                whh[ki][:, o : o + sz],
                                h[ki][:],
                                start=(ki == 0),
                                stop=(ki == len(KT) - 1),
                            )
                        nc.vector.tensor_add(p[:], p[:], xg[0:sz, mi, :])
                        ht = hpool.tile([sz, BPC], f16, tag=f"h{mi}")
                        nc.scalar.activation(
                            ht[:], p[:], Tanh, bias=bias_t[0:sz, mi : mi + 1]
                        )
                        hn.append(ht)
                h = hn

            # ---- MLP head ----
            h1 = []
            for mi in range(2):
                p = ppool.tile([128, BPC], f32, tag=f"p{mi}")
                for ki, (ko, ks) in enumerate(KT):
                    nc.tensor.matmul(
                        p[:],
                        fc1[ki][:, mi * 128 : (mi + 1) * 128],
                        h[ki][:],
                        start=(ki == 0),
                        stop=(ki == len(KT) - 1),
                    )
                ht = hpool.tile([128, BPC], f16, tag=f"h1_{mi}")
                nc.scalar.activation(
                    ht[:], p[:], Relu, bias=fc1b_t[:, mi : mi + 1]
                )
                h1.append(ht)
            p2 = ppool.tile([3, BPC], f32, tag="p2")
            nc.tensor.matmul(p2[:], fc2[0][:, :], h1[0][:], start=True, stop=False)
            nc.tensor.matmul(p2[:], fc2[1][:, :], h1[1][:], start=False, stop=True)
            osb = cpool.tile([3, BPC], f32, tag="osb")
            nc.vector.tensor_scalar_add(osb[:], p2[:], fc2b_t[:, 0:1])
            nc.sync.dma_start(out.ap(), osb[:])

    mybir.codegen_inst_isa_subclasses(nc)
    if split_multiwait:
        _split_multiwait(nc, mybir)
    return nc


def _prep_inputs_v2(x, emb, w_ih, w_hh, b_ih, b_hh, fc1_w, fc1_b, fc2_w, fc2_b,
                    seq=SEQ):
    """Marshal inputs for _build_v2 (layout/dtype only)."""
    x = np.asarray(x)
    assert x.shape == (seq, BATCH), x.shape

    embT = np.ascontiguousarray(np.asarray(emb, np.float16).T)  # [EMB, VOCAB]
    whh_t = np.ascontiguousarray(np.asarray(w_hh, np.float16).T)
    wih_t = np.ascontiguousarray(np.asarray(w_ih, np.float16).T)
    fc1_t = np.ascontiguousarray(np.asarray(fc1_w, np.float16).T)
    fc2_t = np.ascontiguousarray(np.asarray(fc2_w, np.float16).T)

    bias = np.zeros(384, np.float32)
    bias[:HID] = np.asarray(b_ih, np.float32) + np.asarray(b_hh, np.float32)
    bias_sb = np.ascontiguousarray(bias.reshape(3, 128).T)
    fc1b_sb = np.ascontiguousarray(
        np.asarray(fc1_b, np.float32).reshape(2, 128).T
    )
    fc2b_sb = np.asarray(fc2_b, np.float32).reshape(3, 1)

    shared = {
        "embT": embT,
        "whh_t": whh_t,
        "wih_t": wih_t,
        "fc1_t": fc1_t,
        "fc2_t": fc2_t,
        "bias_sb": bias_sb,
        "fc1b_sb": fc1b_sb,
        "fc2b_sb": fc2b_sb,
        "ident": np.eye(128, dtype=np.float16),
    }
    in_maps = []
    for c in range(N_CORES):
        xc = x[:, c * BPC : (c + 1) * BPC]
        flat = np.ascontiguousarray(xc).reshape(-1).astype(np.int16)
        block = np.ascontiguousarray(flat.reshape(-1, 16).T)
        x_idx = np.ascontiguousarray(np.tile(block, (8, 1)))
        in_maps.append({"x_idx": x_idx, **shared})
    return in_maps


def _prep_inputs(x, emb, w_ih, w_hh, b_ih, b_hh, fc1_w, fc1_b, fc2_w, fc2_b,
                 seq=SEQ):
    """Marshal the model inputs into per-core DRAM input maps."""
    x = np.asarray(x)
    assert x.shape == (seq, BATCH), x.shape

    # Embedding table in SBUF-gather layout: partition = token % 128,
    # rank = token // 128, 128 fp16 elems (256B) per row.
    emb_pad = np.zeros((N_RANKS * 128, 128), np.float16)
    emb_pad[:VOCAB, :EMB] = np.asarray(emb, np.float16)
    emb_sb = np.ascontiguousarray(
        emb_pad.reshape(N_RANKS, 128, 128).transpose(1, 0, 2).reshape(128, -1)
    )

    whh_t = np.ascontiguousarray(np.asarray(w_hh, np.float16).T)  # [in, out]
    wih_t = np.ascontiguousarray(np.asarray(w_ih, np.float16).T)  # [emb, hid]
    fc1_t = np.ascontiguousarray(np.asarray(fc1_w, np.float16).T)  # [hid, 256]
    fc2_t = np.ascontiguousarray(np.asarray(fc2_w, np.float16).T)  # [256, 3]

    bias = np.zeros(384, np.float32)
    bias[:HID] = np.asarray(b_ih, np.float32) + np.asarray(b_hh, np.float32)
    bias_sb = np.ascontiguousarray(bias.reshape(3, 128).T)  # [128, 3]
    fc1b_sb = np.ascontiguousarray(
        np.asarray(fc1_b, np.float32).reshape(2, 128).T
    )
    fc2b_sb = np.asarray(fc2_b, np.float32).reshape(3, 1)

    shared = {
        "emb_sb": emb_sb,
        "whh_t": whh_t,
        "wih_t": wih_t,
        "fc1_t": fc1_t,
        "fc2_t": fc2_t,
        "bias_sb": bias_sb,
        "fc1b_sb": fc1b_sb,
        "fc2b_sb": fc2b_sb,
    }
    in_maps = []
    for c in range(N_CORES):
        xc = x[:, c * BPC : (c + 1) * BPC]  # [seq, 512]
        flat = np.ascontiguousarray(xc).reshape(-1).astype(np.int16)
        block = np.ascontiguousarray(flat.reshape(-1, 16).T)  # [16, seq*BPC/16]
        x_idx = np.ascontiguousarray(np.tile(block, (8, 1)))  # [128, ...]
        in_maps.append({"x_idx": x_idx, **shared})
    return in_maps


# Shipping configuration: the K-outer v1 structure. Measured on HW via
# pipelined batch-slope benching: 4.76us/step; all restructurings (table
# pre-projection, DVE/identity xp-injection, M-outer ordering, fp8, PE
# row-packing) measured equal or worse on hardware.
BEST_BUILD_KWARGS = {}


def _get_nc():
    if "nc" not in _cached:
        _cached["nc"] = _build(**BEST_BUILD_KWARGS)
    return _cached["nc"]


def _prep(x, emb, w_ih, w_hh, b_ih, b_hh, fc1_w, fc1_b, fc2_w, fc2_b):
    return _prep_inputs(
        x, emb, w_ih, w_hh, b_ih, b_hh, fc1_w, fc1_b, fc2_w, fc2_b
    )


def kernel(x, emb, w_ih, w_hh, b_ih, b_hh, fc1_w, fc1_b, fc2_w, fc2_b):
    from concourse.bass_utils import run_bass_kernel_spmd

    nc = _get_nc()
    in_maps = _prep(
        x, emb, w_ih, w_hh, b_ih, b_hh, fc1_w, fc1_b, fc2_w, fc2_b
    )
    res = run_bass_kernel_spmd(nc, in_maps, core_ids=list(range(N_CORES)))
    # per-core out is [3, 512]; assemble full [4096, 3]
    full = np.concatenate([r["out"].T for r in res.results], axis=0)
    return full.astype(np.float32)



# revision 34
# speedup vs baseline: 1.0917x; 1.0917x over previous
"""Trainium2 Bass kernel for nn_Net_34729105555716.

Model: embedding lookup [30000,100] -> input projection (w_ih) -> 200-step
tanh RNN (hidden 300) -> relu MLP (300->256->3) over batch 4096.

Strategy (data-parallel over batch, 512 rows per core, 8 cores):
  - The fp16 embedding table lives in SBUF in dma_gather's
    tokens_per_rank=128 layout (partition = token%128, rank = token//128,
    rows padded 100->128 elems = 256B).
  - Per RNN step, a transpose-mode SWDGE dma_gather pulls the 512 tokens'
    embedding rows directly into matmul rhs layout [emb_dim(partitions),
    batch(free)], batched 2 steps per gather instruction.
  - PE does 12 fp16 matmuls per step (3 input-projection + 9 recurrent,
    K-tiles 128/128/44, M-tiles 128/128/44, N=512) accumulating in PSUM.
  - ScalarE applies tanh with the per-hidden bias (b_ih+b_hh) fused,
    writing the fp16 hidden state for the next step.
  - MLP head: 6 + 2 matmuls, relu fused on ScalarE, fc2 bias on VectorE.
Host side only marshals inputs (dtype cast, transpose, index layout) and
transposes the [3,512] per-core outputs back to [4096,3].
"""

import sys

if "/opt/trn_rl_repo" not in sys.path:
    sys.path.insert(0, "/opt/trn_rl_repo")

import numpy as np

SEQ = 200
BATCH = 4096
VOCAB = 30000
EMB = 100
HID = 300
FC1 = 256
N_CORES = 8
BPC = BATCH // N_CORES  # batch per core
G = 1  # RNN steps per dma_gather instruction (>512 idxs per gather crashes)
N_RANKS = (VOCAB + 127) // 128  # 235
KT = [(0, 128), (128, 128), (256, 44)]  # hidden-dim tiles

_cached = {}


def _split_multiwait(nc, mybir):
    """walrus in this container rejects >1 embedded sync wait per
    instruction (>2 for EventSemaphore); split extras onto NoOp carriers."""
    n = 0
    for f in nc.m.functions:
        for blk in f.blocks:
            if not any(
                i.sync_info is not None and len(i.sync_info.on_wait) > 1
                for i in blk.instructions
            ):
                continue
            out = []
            for inst in blk.instructions:
                si = inst.sync_info
                cap = 2 if isinstance(inst, mybir.InstEventSemaphore) else 1
                if si is not None and len(si.on_wait) > cap:
                    waits = list(si.on_wait)
                    for w in waits[:-cap]:
                        n += 1
                        carrier = mybir.InstNoOp(
                            name=f"I-waitsplit-{n}", ins=[], outs=[]
                        )
                        carrier.engine = inst.engine
                        carrier.sync_info = mybir.SyncInfo(
                            on_wait=[w], on_update=[]
                        )
                        out.append(carrier)
                    si.on_wait = waits[-cap:]
                out.append(inst)
            blk.instructions = out
    return n


def _build(seq=SEQ, split_multiwait=True, no_gather=False, mouter=False,
           cw=False):
    import concourse.bass as bass
    import concourse.mybir as mybir
    import concourse.tile as tile
    from concourse import library_config
    from concourse.tile import add_dep_helper

    dt = mybir.dt
    f16, f32, i16 = dt.float16, dt.float32, dt.int16
    Tanh = mybir.ActivationFunctionType.Tanh
    Relu = mybir.ActivationFunctionType.Relu

    nc = bass.Bass(
        "TRN2", target_bir_lowering=False, debug=False, num_devices=N_CORES
    )
    x_idx = nc.dram_tensor(
        "x_idx", [128, seq * BPC // 16], i16, kind="ExternalInput"
    )
    emb_sb = nc.dram_tensor(
        "emb_sb", [128, N_RANKS * 128], f16, kind="ExternalInput"
    )
    whh_t = nc.dram_tensor("whh_t", [HID, HID], f16, kind="ExternalInput")
    wih_t = nc.dram_tensor("wih_t", [EMB, HID], f16, kind="ExternalInput")
    fc1_t = nc.dram_tensor("fc1_t", [HID, FC1], f16, kind="ExternalInput")
    fc2_t = nc.dram_tensor("fc2_t", [FC1, 3], f16, kind="ExternalInput")
    bias_sb = nc.dram_tensor("bias_sb", [128, 3], f32, kind="ExternalInput")
    fc1b_sb = nc.dram_tensor("fc1b_sb", [128, 2], f32, kind="ExternalInput")
    fc2b_sb = nc.dram_tensor("fc2b_sb", [3, 1], f32, kind="ExternalInput")
    out = nc.dram_tensor("out", [3, BPC], f32, kind="ExternalOutput")

    with tile.TileContext(nc) as tc:
        with (
            tc.tile_pool(name="const", bufs=1) as cpool,
            tc.tile_pool(name="gather", bufs=4) as gpool,
            tc.tile_pool(name="h", bufs=2) as hpool,
            tc.tile_pool(name="psum", bufs=2, space="PSUM") as ppool,
        ):
            lib_inst = nc.gpsimd.load_library(library_config.mlp)

            tbl = cpool.tile([128, N_RANKS * 128], f16, tag="tbl")
            nc.sync.dma_start(tbl[:], emb_sb.ap())
            idx = cpool.tile([128, seq * BPC // 16], i16, tag="idx")
            nc.sync.dma_start(idx[:], x_idx.ap())

            whh = []
            for o, sz in KT:
                w = cpool.tile([sz, HID], f16, tag=f"whh{o}")
                nc.sync.dma_start(w[:], whh_t.ap()[o : o + sz, :])
                whh.append(w)
            wih = cpool.tile([EMB, HID], f16, tag="wih")
            nc.sync.dma_start(wih[:], wih_t.ap())
            if cw:
                # contiguous per-(K,M) weight tiles: a 128-wide lhsT whose
                # rows are contiguous enables the compiler's fast-weight-load
                # path (strided column slices of a 300-wide tile do not)
                whhc = {}
                for ki, (ko, ks) in enumerate(KT):
                    for mi, (mo, ms) in enumerate(KT):
                        w = cpool.tile([ks, ms], f16, tag=f"whhc{ki}_{mi}")
                        nc.sync.dma_start(
                            w[:], whh_t.ap()[ko : ko + ks, mo : mo + ms]
                        )
                        whhc[(ki, mi)] = w
                wihc = {}
                for mi, (mo, ms) in enumerate(KT):
                    w = cpool.tile([EMB, ms], f16, tag=f"wihc{mi}")
                    nc.sync.dma_start(w[:], wih_t.ap()[:, mo : mo + ms])
                    wihc[mi] = w
            fc1 = []
            for o, sz in KT:
                w = cpool.tile([sz, FC1], f16, tag=f"fc1{o}")
                nc.sync.dma_start(w[:], fc1_t.ap()[o : o + sz, :])
                fc1.append(w)
            if cw:
                fc1c = {}
                for ki, (ko, ks) in enumerate(KT):
                    for mi in range(2):
                        w = cpool.tile([ks, 128], f16, tag=f"fc1c{ki}_{mi}")
                        nc.sync.dma_start(
                            w[:],
                            fc1_t.ap()[ko : ko + ks, mi * 128 : (mi + 1) * 128],
                        )
                        fc1c[(ki, mi)] = w
            fc2 = []
            for o in (0, 128):
                w = cpool.tile([128, 3], f16, tag=f"fc2{o}")
                nc.sync.dma_start(w[:], fc2_t.ap()[o : o + 128, :])
                fc2.append(w)
            bias_t = cpool.tile([128, 3], f32, tag="bias")
            nc.sync.dma_start(bias_t[:], bias_sb.ap())
            fc1b_t = cpool.tile([128, 2], f32, tag="fc1b")
            nc.sync.dma_start(fc1b_t[:], fc1b_sb.ap())
            fc2b_t = cpool.tile([3, 1], f32, tag="fc2b")
            nc.sync.dma_start(fc2b_t[:], fc2b_sb.ap())

            reg_n = nc.gpsimd.to_reg(G * BPC)

            h = []
            for mi, (o, sz) in enumerate(KT):
                ht = hpool.tile([sz, BPC], f16, tag=f"h{mi}")
                nc.vector.memset(ht[:], 0)
                h.append(ht)

            xg = None
            if no_gather:
                xg_fixed = cpool.tile([128, 1, G * BPC], f16, tag="xe_fixed")
                nc.vector.memset(xg_fixed[:], 0)
            for t in range(seq):
                g, sl = divmod(t, G)
                if no_gather:
                    xg = xg_fixed
                elif sl == 0:
                    n_idx = min(G, seq - t) * BPC
                    xg = gpool.tile([128, 1, G * BPC], f16, tag="xe")
                    gi = nc.gpsimd.dma_gather(
                        xg[:, :, :n_idx],
                        tbl[:],
                        idx[:, g * (G * BPC // 16) : g * (G * BPC // 16) + n_idx // 16],
                        n_idx,
                        reg_n,
                        128,
                        transpose=True,
                        sbuf_tokens_per_rank=128,
                        sbuf_free_dim_per_rank=256,
                    )
                    add_dep_helper(
                        gi.ins, lib_inst.ins, sync=False, reason="lib first"
                    )
                xe = xg[0:EMB, 0, sl * BPC : (sl + 1) * BPC]
                if mouter:
                    # M-outer: each psum finishes early (slots 4/8/12), so
                    # each tanh gets pipeline slack before the next step
                    # needs its h tile
                    hn = []
                    for mi, (o, sz) in enumerate(KT):
                        p = ppool.tile([sz, BPC], f32, tag=f"p{mi}")
                        nc.tensor.matmul(
                            p[:], wih[:, o : o + sz], xe, start=True, stop=False
                        )
                        for ki, (ko, ks) in enumerate(KT):
                            nc.tensor.matmul(
                                p[:],
                                whh[ki][:, o : o + sz],
                                h[ki][:],
                                start=False,
                                stop=(ki == len(KT) - 1),
                            )
                        ht = hpool.tile([sz, BPC], f16, tag=f"h{mi}")
                        nc.scalar.activation(
                            ht[:], p[:], Tanh, bias=bias_t[0:sz, mi : mi + 1]
                        )
                        hn.append(ht)
                    h = hn
                    continue
                ps = []
                for mi, (o, sz) in enumerate(KT):
                    p = ppool.tile([sz, BPC], f32, tag=f"p{mi}")
                    nc.tensor.matmul(
                        p[:],
                        wihc[mi][:] if cw else wih[:, o : o + sz],
                        xe,
                        start=True,
                        stop=False,
                    )
                    ps.append(p)
                for ki, (ko, ks) in enumerate(KT):
                    last = ki == len(KT) - 1
                    for mi, (mo, ms) in enumerate(KT):
                        nc.tensor.matmul(
                            ps[mi][:],
                            whhc[(ki, mi)][:] if cw else whh[ki][:, mo : mo + ms],
                            h[ki][:],
                            start=False,
                            stop=last,
                        )
                hn = []
                for mi, (o, sz) in enumerate(KT):
                    ht = hpool.tile([sz, BPC], f16, tag=f"h{mi}")
                    nc.scalar.activation(
                        ht[:], ps[mi][:], Tanh, bias=bias_t[0:sz, mi : mi + 1]
                    )
                    hn.append(ht)
                h = hn

            # MLP head
            h1 = []
            for mi in range(2):
                p = ppool.tile([128, BPC], f32, tag=f"p{mi}")
                for ki, (ko, ks) in enumerate(KT):
                    nc.tensor.matmul(
                        p[:],
                        fc1c[(ki, mi)][:] if cw else
                        fc1[ki][:, mi * 128 : (mi + 1) * 128],
                        h[ki][:],
                        start=(ki == 0),
                        stop=(ki == len(KT) - 1),
                    )
                ht = hpool.tile([128, BPC], f16, tag=f"h1_{mi}")
                nc.scalar.activation(
                    ht[:], p[:], Relu, bias=fc1b_t[:, mi : mi + 1]
                )
                h1.append(ht)
            p2 = ppool.tile([3, BPC], f32, tag="p2")
            nc.tensor.matmul(p2[:], fc2[0][:, :], h1[0][:], start=True, stop=False)
            nc.tensor.matmul(p2[:], fc2[1][:, :], h1[1][:], start=False, stop=True)
            osb = cpool.tile([3, BPC], f32, tag="osb")
            nc.vector.tensor_scalar_add(osb[:], p2[:], fc2b_t[:, 0:1])
            nc.sync.dma_start(out.ap(), osb[:])

    # Populate .instr bytes for InstISA subclasses (library reload etc.) —
    # Bacc.compile does this; raw Bass+Tile must do it explicitly or walrus
    # fails with "ISA wrong length".
    mybir.codegen_inst_isa_subclasses(nc)
    if split_multiwait:
        _split_multiwait(nc, mybir)
    return nc


def _build_v2(seq=SEQ, split_multiwait=True, no_gather=False, pack_k2=False,
              gq=1, probe=None, fold=False):
    """v2: pre-project the embedding table on device (proj = emb @ w_ih.T,
    [VOCAB, 300] fp16 in SWDGE gather layout), then each RNN step gathers the
    projected rows directly — the 3 input-projection matmuls per step are
    gone (12 -> 9 MMs). M-outer accumulation order gives each tanh ~3 MM
    slots of pipeline slack. Gathered projection is added into PSUM by DVE.
    """
    import concourse.bass as bass
    import concourse.mybir as mybir
    import concourse.tile as tile
    from concourse import library_config
    from concourse.tile import add_dep_helper

    dt = mybir.dt
    f16, f32, i16 = dt.float16, dt.float32, dt.int16
    Tanh = mybir.ActivationFunctionType.Tanh
    Relu = mybir.ActivationFunctionType.Relu
    Copy = mybir.ActivationFunctionType.Copy
    STRIPE = 384  # fp16 elems per proj rank stripe (300 used + 84 pad), 768B
    IC = 16  # idx chunk: steps per streamed idx tile

    nc = bass.Bass(
        "TRN2", target_bir_lowering=False, debug=False, num_devices=N_CORES,
        num_swdge_queues=gq,
    )
    x_idx = nc.dram_tensor(
        "x_idx", [128, seq * BPC // 16], i16, kind="ExternalInput"
    )
    embT = nc.dram_tensor("embT", [EMB, VOCAB], f16, kind="ExternalInput")
    whh_t = nc.dram_tensor("whh_t", [HID, HID], f16, kind="ExternalInput")
    wih_t = nc.dram_tensor("wih_t", [EMB, HID], f16, kind="ExternalInput")
    fc1_t = nc.dram_tensor("fc1_t", [HID, FC1], f16, kind="ExternalInput")
    fc2_t = nc.dram_tensor("fc2_t", [FC1, 3], f16, kind="ExternalInput")
    bias_sb = nc.dram_tensor("bias_sb", [128, 3], f32, kind="ExternalInput")
    fc1b_sb = nc.dram_tensor("fc1b_sb", [128, 2], f32, kind="ExternalInput")
    fc2b_sb = nc.dram_tensor("fc2b_sb", [3, 1], f32, kind="ExternalInput")
    ident = None
    if fold:
        ident = nc.dram_tensor("ident", [128, 128], f16, kind="ExternalInput")
    out = nc.dram_tensor("out", [3, BPC], f32, kind="ExternalOutput")

    n_chunks = (VOCAB + 127) // 128  # 235, last chunk 48 tokens

    with tile.TileContext(nc) as tc:
        with (
            tc.tile_pool(name="const", bufs=1) as cpool,
            tc.tile_pool(name="estream", bufs=2) as epool,
            tc.tile_pool(name="idxs", bufs=2) as ipool,
            tc.tile_pool(name="gather", bufs=2) as gpool,
            tc.tile_pool(name="h", bufs=2) as hpool,
            tc.tile_pool(name="psum", bufs=2, space="PSUM") as ppool,
            tc.tile_pool(name="pprep", bufs=2, space="PSUM") as prep_pool,
        ):
            lib_inst = nc.gpsimd.load_library(library_config.mlp)

            whh = []
            for o, sz in KT:
                w = cpool.tile([sz, HID], f16, tag=f"whh{o}")
                nc.sync.dma_start(w[:], whh_t.ap()[o : o + sz, :])
                whh.append(w)
            wih = cpool.tile([EMB, HID], f16, tag="wih")
            nc.sync.dma_start(wih[:], wih_t.ap())
            fc1 = []
            for o, sz in KT:
                w = cpool.tile([sz, FC1], f16, tag=f"fc1{o}")
                nc.sync.dma_start(w[:], fc1_t.ap()[o : o + sz, :])
                fc1.append(w)
            fc2 = []
            for o in (0, 128):
                w = cpool.tile([128, 3], f16, tag=f"fc2{o}")
                nc.sync.dma_start(w[:], fc2_t.ap()[o : o + 128, :])
                fc2.append(w)
            bias_t = cpool.tile([128, 3], f32, tag="bias")
            nc.sync.dma_start(bias_t[:], bias_sb.ap())
            fc1b_t = cpool.tile([128, 2], f32, tag="fc1b")
            nc.sync.dma_start(fc1b_t[:], fc1b_sb.ap())
            fc2b_t = cpool.tile([3, 1], f32, tag="fc2b")
            nc.sync.dma_start(fc2b_t[:], fc2b_sb.ap())
            if fold:
                ident_t = cpool.tile([128, 128], f16, tag="ident")
                nc.sync.dma_start(ident_t[:], ident.ap())

            # ---- phase A: project the embedding table ----
            proj = cpool.tile([128, n_chunks, STRIPE], f16, tag="proj")
            # zero the stripe pads (gather copies whole 768B rows) and the
            # tail of the last chunk (tokens 29952..30080 don't all exist)
            nc.vector.memset(proj[:, :, HID:STRIPE], 0)
            nc.vector.memset(proj[32:64, n_chunks - 1, 0:HID], 0)
            nc.vector.memset(proj[64:128, n_chunks - 1, 0:HID], 0)
            CH = 4  # table chunks per DMA load
            for c0 in range(0, n_chunks, CH):
                nch = min(CH, n_chunks - c0)
                ncol = min(nch * 128, VOCAB - c0 * 128)
                ech = epool.tile([EMB, CH * 128], f16, tag="ech")
                nc.sync.dma_start(
                    ech[:, :ncol], embT.ap()[:, c0 * 128 : c0 * 128 + ncol]
                )
                for c in range(c0, c0 + nch):
                    ntok = min(128, VOCAB - c * 128)
                    off = (c - c0) * 128
                    pp = prep_pool.tile([128, HID], f32, tag="pp")
                    nc.tensor.matmul(
                        pp[:ntok, :],
                        ech[:, off : off + ntok],
                        wih[:],
                        start=True,
                        stop=True,
                    )
                    dst = proj[:ntok, c, 0:HID]
                    if c % 2 == 0:
                        nc.vector.tensor_copy(dst, pp[:ntok, :])
                    else:
                        nc.scalar.activation(dst, pp[:ntok, :], Copy)

            reg_n = nc.gpsimd.to_reg(BPC)

            if pack_k2:
                # duplicate of whh k2-rows at partitions 64..107 so the
                # m1/k2 matmul can run in the 64-127 row strips of the PE
                # array concurrently with m0/k2 in rows 0-63
                whh2b_t = cpool.tile([128, HID], f16, tag="whh2b")
                nc.sync.dma_start(
                    whh2b_t[64 : 64 + 44, :], whh_t.ap()[256:300, :]
                )

            h = []
            for mi, (o, sz) in enumerate(KT):
                ht = hpool.tile([sz, BPC], f16, tag=f"h{mi}")
                nc.vector.memset(ht[:], 0)
                h.append(ht)
            h2b = None
            if pack_k2:
                h2b_t = hpool.tile([128, BPC], f16, tag="h2b")
                nc.vector.memset(h2b_t[64 : 64 + 44, :], 0)
                h2b = h2b_t

            if no_gather:
                xg_fixed = cpool.tile([128, 3, BPC], f16, tag="xg_fixed")
                nc.vector.memset(xg_fixed[:], 0)

            # ---- phase B: recurrent steps ----
            idx_t = None
            for t in range(seq):
                ci, sl = divmod(t, IC)
                if sl == 0:
                    nst = min(IC, seq - ci * IC)
                    idx_t = ipool.tile([128, IC * BPC // 16], i16, tag="idx")
                    nc.sync.dma_start(
                        idx_t[:, : nst * BPC // 16],
                        x_idx.ap()[
                            :, ci * IC * BPC // 16 : (ci * IC + nst) * BPC // 16
                        ],
                    )
                if no_gather:
                    xg = xg_fixed
                else:
                    xg = gpool.tile([128, 3, BPC], f16, tag="xg")
                    gi = nc.gpsimd.dma_gather(
                        xg[:],
                        proj[:],
                        idx_t[:, sl * (BPC // 16) : (sl + 1) * (BPC // 16)],
                        BPC,
                        reg_n,
                        STRIPE,
                        transpose=True,
                        sbuf_tokens_per_rank=128,
                        sbuf_free_dim_per_rank=2 * STRIPE,
                        queue_num=t % gq,
                    )
                    add_dep_helper(
                        gi.ins, lib_inst.ins, sync=False, reason="lib first"
                    )
                hn = []
                if pack_k2:
                    # 8 PE slots: m0(k0,k1,k2) with m1k2 packed beside m0k2
                    # (rows 64-127), then m1(k0,k1), then m2(k0,k1,k2)
                    ps = []
                    for mi, (o, sz) in enumerate(KT):
                        p = ppool.tile([sz, BPC], f32, tag=f"p{mi}")
                        ps.append(p)
                    nc.tensor.matmul(
                        ps[0][:], whh[0][:, 0:128], h[0][:],
                        start=True, stop=False,
                    )
                    nc.tensor.matmul(
                        ps[0][:], whh[1][:, 0:128], h[1][:],
                        start=False, stop=False,
                    )
                    nc.tensor.matmul(
                        ps[0][:], whh[2][:, 0:128], h[2][:],
                        start=False, stop=True,
                    )
                    nc.tensor.matmul(
                        ps[1][:], whh2b_t[64 : 64 + 44, 128:256],
                        h2b[64 : 64 + 44, :], start=True, stop=False,
                    )
                    nc.tensor.matmul(
                        ps[1][:], whh[0][:, 128:256], h[0][:],
                        start=False, stop=False,
                    )
                    nc.tensor.matmul(
                        ps[1][:], whh[1][:, 128:256], h[1][:],
                        start=False, stop=True,
                    )
                    for ki in range(3):
                        nc.tensor.matmul(
                            ps[2][:], whh[ki][:, 256:300], h[ki][:],
                            start=(ki == 0), stop=(ki == 2),
                        )
                    for mi, (o, sz) in enumerate(KT):
                        nc.vector.tensor_add(
                            ps[mi][:], ps[mi][:], xg[0:sz, mi, :]
                        )
                        ht = hpool.tile([sz, BPC], f16, tag=f"h{mi}")
                        nc.scalar.activation(
                            ht[:], ps[mi][:], Tanh,
                            bias=bias_t[0:sz, mi : mi + 1],
                        )
                        hn.append(ht)
                    h2b = hpool.tile([128, BPC], f16, tag="h2b")
                    nc.sync.dma_start(h2b[64 : 64 + 44, :], hn[2][:])
                elif probe == "mm":
                    # PE stream only: no adds, no tanh, h stays static
                    for mi, (o, sz) in enumerate(KT):
                        p = ppool.tile([sz, BPC], f32, tag=f"p{mi}")
                        for ki, (ko, ks) in enumerate(KT):
                            nc.tensor.matmul(
                                p[:],
                                whh[ki][:, o : o + sz],
                                h[ki][:],
                                start=(ki == 0),
                                stop=(ki == len(KT) - 1),
                            )
                    hn = h
                elif probe == "mmact":
                    # PE + ScalarE in parallel, but tanh output unused
                    for mi, (o, sz) in enumerate(KT):
                        p = ppool.tile([sz, BPC], f32, tag=f"p{mi}")
                        for ki, (ko, ks) in enumerate(KT):
                            nc.tensor.matmul(
                                p[:],
                                whh[ki][:, o : o + sz],
                                h[ki][:],
                                start=(ki == 0),
                                stop=(ki == len(KT) - 1),
                            )
                        ht = hpool.tile([sz, BPC], f16, tag=f"dump{mi}")
                        nc.scalar.activation(
                            ht[:], p[:], Tanh, bias=bias_t[0:sz, mi : mi + 1]
                        )
                    hn = h
                elif fold:
                    # inject xp into psum via identity matmul (slot 1 of each
                    # M-block, no h dependency) -> tanh reads psum directly;
                    # 2-hop loop chain, DVE stays out of it
                    for mi, (o, sz) in enumerate(KT):
                        p = ppool.tile([sz, BPC], f32, tag=f"p{mi}")
                        nc.tensor.matmul(
                            p[:],
                            ident_t[0:sz, 0:sz],
                            xg[0:sz, mi, :],
                            start=True,
                            stop=False,
                        )
                        for ki, (ko, ks) in enumerate(KT):
                            nc.tensor.matmul(
                                p[:],
                                whh[ki][:, o : o + sz],
                                h[ki][:],
                                start=False,
                                stop=(ki == len(KT) - 1),
                            )
                        ht = hpool.tile([sz, BPC], f16, tag=f"h{mi}")
                        nc.scalar.activation(
                            ht[:], p[:], Tanh, bias=bias_t[0:sz, mi : mi + 1]
                        )
                        hn.append(ht)
                else:
                    for mi, (o, sz) in enumerate(KT):
                        p = ppool.tile([sz, BPC], f32, tag=f"p{mi}")
                        for ki, (ko, ks) in enumerate(KT):
                            nc.tensor.matmul(
                                p[:],
                                whh[ki][:, o : o + sz],
                                h[ki][:],
                                start=(ki == 0),
                                stop=(ki == len(KT) - 1),
                            )
                        nc.vector.tensor_add(p[:], p[:], xg[0:sz, mi, :])
                        ht = hpool.tile([sz, BPC], f16, tag=f"h{mi}")
                        nc.scalar.activation(
                            ht[:], p[:], Tanh, bias=bias_t[0:sz, mi : mi + 1]
                        )
                        hn.append(ht)
                h = hn

            # ---- MLP head ----
            h1 = []
            for mi in range(2):
                p = ppool.tile([128, BPC], f32, tag=f"p{mi}")
                for ki, (ko, ks) in enumerate(KT):
                    nc.tensor.matmul(
                        p[:],
                        fc1[ki][:, mi * 128 : (mi + 1) * 128],
                        h[ki][:],
                        start=(ki == 0),
                        stop=(ki == len(KT) - 1),
                    )
                ht = hpool.tile([128, BPC], f16, tag=f"h1_{mi}")
                nc.scalar.activation(
                    ht[:], p[:], Relu, bias=fc1b_t[:, mi : mi + 1]
                )
                h1.append(ht)
            p2 = ppool.tile([3, BPC], f32, tag="p2")
            nc.tensor.matmul(p2[:], fc2[0][:, :], h1[0][:], start=True, stop=False)
            nc.tensor.matmul(p2[:], fc2[1][:, :], h1[1][:], start=False, stop=True)
            osb = cpool.tile([3, BPC], f32, tag="osb")
            nc.vector.tensor_scalar_add(osb[:], p2[:], fc2b_t[:, 0:1])
            nc.sync.dma_start(out.ap(), osb[:])

    mybir.codegen_inst_isa_subclasses(nc)
    if split_multiwait:
        _split_multiwait(nc, mybir)
    return nc


def _prep_inputs_v2(x, emb, w_ih, w_hh, b_ih, b_hh, fc1_w, fc1_b, fc2_w, fc2_b,
                    seq=SEQ):
    """Marshal inputs for _build_v2 (layout/dtype only)."""
    x = np.asarray(x)
    assert x.shape == (seq, BATCH), x.shape

    embT = np.ascontiguousarray(np.asarray(emb, np.float16).T)  # [EMB, VOCAB]
    whh_t = np.ascontiguousarray(np.asarray(w_hh, np.float16).T)
    wih_t = np.ascontiguousarray(np.asarray(w_ih, np.float16).T)
    fc1_t = np.ascontiguousarray(np.asarray(fc1_w, np.float16).T)
    fc2_t = np.ascontiguousarray(np.asarray(fc2_w, np.float16).T)

    bias = np.zeros(384, np.float32)
    bias[:HID] = np.asarray(b_ih, np.float32) + np.asarray(b_hh, np.float32)
    bias_sb = np.ascontiguousarray(bias.reshape(3, 128).T)
    fc1b_sb = np.ascontiguousarray(
        np.asarray(fc1_b, np.float32).reshape(2, 128).T
    )
    fc2b_sb = np.asarray(fc2_b, np.float32).reshape(3, 1)

    shared = {
        "embT": embT,
        "whh_t": whh_t,
        "wih_t": wih_t,
        "fc1_t": fc1_t,
        "fc2_t": fc2_t,
        "bias_sb": bias_sb,
        "fc1b_sb": fc1b_sb,
        "fc2b_sb": fc2b_sb,
        "ident": np.eye(128, dtype=np.float16),
    }
    in_maps = []
    for c in range(N_CORES):
        xc = x[:, c * BPC : (c + 1) * BPC]
        flat = np.ascontiguousarray(xc).reshape(-1).astype(np.int16)
        block = np.ascontiguousarray(flat.reshape(-1, 16).T)
        x_idx = np.ascontiguousarray(np.tile(block, (8, 1)))
        in_maps.append({"x_idx": x_idx, **shared})
    return in_maps


def _prep_inputs(x, emb, w_ih, w_hh, b_ih, b_hh, fc1_w, fc1_b, fc2_w, fc2_b,
                 seq=SEQ):
    """Marshal the model inputs into per-core DRAM input maps."""
    x = np.asarray(x)
    assert x.shape == (seq, BATCH), x.shape

    # Embedding table in SBUF-gather layout: partition = token % 128,
    # rank = token // 128, 128 fp16 elems (256B) per row.
    emb_pad = np.zeros((N_RANKS * 128, 128), np.float16)
    emb_pad[:VOCAB, :EMB] = np.asarray(emb, np.float16)
    emb_sb = np.ascontiguousarray(
        emb_pad.reshape(N_RANKS, 128, 128).transpose(1, 0, 2).reshape(128, -1)
    )

    whh_t = np.ascontiguousarray(np.asarray(w_hh, np.float16).T)  # [in, out]
    wih_t = np.ascontiguousarray(np.asarray(w_ih, np.float16).T)  # [emb, hid]
    fc1_t = np.ascontiguousarray(np.asarray(fc1_w, np.float16).T)  # [hid, 256]
    fc2_t = np.ascontiguousarray(np.asarray(fc2_w, np.float16).T)  # [256, 3]

    bias = np.zeros(384, np.float32)
    bias[:HID] = np.asarray(b_ih, np.float32) + np.asarray(b_hh, np.float32)
    bias_sb = np.ascontiguousarray(bias.reshape(3, 128).T)  # [128, 3]
    fc1b_sb = np.ascontiguousarray(
        np.asarray(fc1_b, np.float32).reshape(2, 128).T
    )
    fc2b_sb = np.asarray(fc2_b, np.float32).reshape(3, 1)

    shared = {
        "emb_sb": emb_sb,
        "whh_t": whh_t,
        "wih_t": wih_t,
        "fc1_t": fc1_t,
        "fc2_t": fc2_t,
        "bias_sb": bias_sb,
        "fc1b_sb": fc1b_sb,
        "fc2b_sb": fc2b_sb,
    }
    in_maps = []
    for c in range(N_CORES):
        xc = x[:, c * BPC : (c + 1) * BPC]  # [seq, 512]
        flat = np.ascontiguousarray(xc).reshape(-1).astype(np.int16)
        block = np.ascontiguousarray(flat.reshape(-1, 16).T)  # [16, seq*BPC/16]
        x_idx = np.ascontiguousarray(np.tile(block, (8, 1)))  # [128, ...]
        in_maps.append({"x_idx": x_idx, **shared})
    return in_maps


# Shipping configuration: the K-outer v1 structure. Measured on HW via
# pipelined batch-slope benching: 4.76us/step; all restructurings (table
# pre-projection, DVE/identity xp-injection, M-outer ordering, fp8, PE
# row-packing) measured equal or worse on hardware.
BEST_BUILD_KWARGS = {}


def _get_nc():
    if "nc" not in _cached:
        _cached["nc"] = _build(**BEST_BUILD_KWARGS)
    return _cached["nc"]


def _prep(x, emb, w_ih, w_hh, b_ih, b_hh, fc1_w, fc1_b, fc2_w, fc2_b):
    return _prep_inputs(
        x, emb, w_ih, w_hh, b_ih, b_hh, fc1_w, fc1_b, fc2_w, fc2_b
    )


def kernel(x, emb, w_ih, w_hh, b_ih, b_hh, fc1_w, fc1_b, fc2_w, fc2_b):
    from concourse.bass_utils import run_bass_kernel_spmd

    nc = _get_nc()
    in_maps = _prep(
        x, emb, w_ih, w_hh, b_ih, b_hh, fc1_w, fc1_b, fc2_w, fc2_b
    )
    res = run_bass_kernel_spmd(nc, in_maps, core_ids=list(range(N_CORES)))
    # per-core out is [3, 512]; assemble full [4096, 3]
    full = np.concatenate([r["out"].T for r in res.results], axis=0)
    return full.astype(np.float32)

